# revision 45
# baseline (speedup 1.0000x reference)
"""Trainium2 Bass kernel for nn_EntitiesIndexingHeadRuleBased (nms_detection).

kernel(**inputs) takes the FULL batch (B=64) and returns (sub_dist, obj_dist),
each [64, 500, 500] float32, matching the reference semantics:

  out_s[r,e] = relu(giou) * score_e / ((|vx-cx_e|+|vy-cy_e|+1) * (sqrt(d2)+1))

Sharding: pure data parallelism - 8 images per NeuronCore across 8 cores.

v2 redesign (vs f32 baseline):
  * fp16 intermediates unlock DVE 2x/4x perf modes (tensor_scalar 4x,
    tensor_tensor 2x); giou in N-form (relu(C*(I-U)+U^2) / (U*C)) with a
    single ACT-engine reciprocal covering the whole denominator.
  * GIoU evaluated on NORMALIZED boxes (scale-invariant), which keeps all
    box math in [0,2] and skips the W/H scaling of box tensors.
  * pixel centers stay f32; |cx - vx| subtraction runs as f32-input
    tensor_scalar (2x mode) to dodge catastrophic cancellation.
  * cls distance d2 >= 0 enforced structurally: both pack norms are
    sums of the same fp16-rounded prob vectors the matmul sees.
  * work spread across engines: DVE (tensor_scalar/tensor_tensor/divide),
    ACT (exp/sqrt/relu/square), Pool aka GpSimd (fold-ops + f32 output
    convert), PE (transposes + cdist matmuls), SyncE (DMA).
  * map-wide ops run at quad width [125, 2000] (4 chunks fused) to
    amortize per-op overheads.
"""
import sys
sys.path.insert(0, '/opt/trn_rl_repo')

import numpy as np
import bass_rust
import concourse.bass as bass
import concourse.tile as tile
import concourse.tile as tile_mod
from concourse import mybir
from concourse import bass_utils
from concourse.masks import make_identity
from concourse.tile import TileContext

F32 = mybir.dt.float32
F16 = mybir.dt.float16
AF = mybir.ActivationFunctionType
OP = mybir.AluOpType

B = 64
NE = 500
NR = 500
NC1 = 151
NCL = 150
P = 125
NCH = 4
NQ = NCH * NE          # quad width (2000)
N_CORES = 8
N_IMG = B // N_CORES

# ---------------------------------------------------------------------------
# Workarounds for the container's walrus: it rejects instructions carrying
# more than one sync-wait command ("Too many sync wait commands").
# ---------------------------------------------------------------------------

_MAXW = 1


def _patched_drain_and_barrier(self, tick_clock, wait_clock):
    ScopedClock = tile_mod.ScopedClock
    carrier = self.nc.sync.nop(nofuse=True)
    wait_clock.add_sem_waits(carrier.ins,
                             ScopedClock({None: tick_clock.global_clock}))
    si = carrier.ins.sync_info
    waits = list(si.on_wait) if si is not None else []
    if len(waits) > _MAXW:
        carrier.ins.sync_info = bass_rust.SyncInfo(
            on_wait=waits[:_MAXW], on_update=[])
        for i in range(_MAXW, len(waits), _MAXW):
            nop = self.nc.sync.nop(nofuse=True)
            nop.ins.sync_info = bass_rust.SyncInfo(
                on_wait=waits[i:i + _MAXW], on_update=[])
    self.nc.sync.drain()
    self.nc.all_engine_barrier()
    assert self.sems is not None
    popped = self.nc._tile_sem_poison_stack.pop()
    assert popped is self._sem_poison
    self.nc.clear_and_free_semaphores(list(self.sems.allocated().values()))
    self.nc.all_engine_barrier()


TileContext._drain_and_barrier = _patched_drain_and_barrier


def _split_waits(nc, maxw=_MAXW):
    """Hoist excess sync waits onto same-engine NoOps placed just before the
    offending instruction (engine streams execute in order)."""
    for fn in nc.m.functions:
        for blk in fn.blocks:
            newl = []
            changed = False
            for ins in blk.instructions:
                si = ins.sync_info
                waits = list(si.on_wait) if si is not None else []
                if len(waits) > maxw:
                    changed = True
                    carried, rest = waits[:-maxw], waits[-maxw:]
                    for i in range(0, len(carried), maxw):
                        nop = mybir.InstNoOp(
                            name=f"{ins.name}-sw{i}",
                            sync_info=mybir.SyncInfo(
                                on_wait=carried[i:i + maxw], on_update=[]),
                            bass_nofuse=True,
                            engine=ins.engine,
                        )
                        newl.append(nop)
                    ins.sync_info = mybir.SyncInfo(
                        on_wait=rest, on_update=list(si.on_update))
                newl.append(ins)
            if changed:
                blk.instructions = newl


# ---------------------------------------------------------------------------
# Custom fused DVE ops (registered into the process-local dve_ops registry;
# the per-NEFF DVE table is generated from these at compile time).
# ---------------------------------------------------------------------------

import numpy as _np
from concourse import dve_ops as _dve_ops
from concourse.dve_spec import (Spec, Src0, Src1, C0, C1, Zero, One,
                                maxx, minn, relu, lower,
                                _has_src1 as _spec_has_src1)
from concourse.dve_uop import DveOpSpec


def _register_op(name, spec):
    for o in _dve_ops.OPS:
        if o.name == name:
            return o
    row = _dve_ops._CUSTOM_DVE_ROW_BASE + len(_dve_ops.OPS)
    assert row < 0x20
    _dve_ops._SUB_OPCODE_FOR_NAME[name] = row
    shas = {}
    for ver in ("v3", "v4"):
        s = DveOpSpec(name=name, opcode=row, uops=lower(spec, ver=ver),
                      rd1_en=_spec_has_src1(spec))
        shas[ver] = s.sha(ver)
    op = _dve_ops.DveOp(name, spec, subdim=False, uops_sha=shas)
    _dve_ops.OPS.append(op)
    _dve_ops.CUSTOM_DVE_SPECS[name] = spec
    return op


# dx = min(in0, s0) - max(in1, s1)
DX_MM = _register_op("ANT_DX_MM", Spec(
    body=minn(Src0, C0) - maxx(Src1, C1),
    reference=lambda in0, in1, s0, s1, imm2:
        _np.minimum(in0, s0) - _np.maximum(in1, s1)))

# L = |in0 + s0| + |in1 + s1|
_a = Src0 + C0
_b = Src1 + C1
L1_AB = _register_op("ANT_L1_AB", Spec(
    body=maxx(_a, Zero - _a) + maxx(_b, Zero - _b),
    reference=lambda in0, in1, s0, s1, imm2:
        _np.abs(in0 + s0) + _np.abs(in1 + s1)))

# d1 = (in0 + 1) * (in1 + 1)
D1_FUSE = _register_op("ANT_D1_FUSE", Spec(
    body=(Src0 + One) * (Src1 + One),
    reference=lambda in0, in1, s0, s1, imm2: (in0 + 1.0) * (in1 + 1.0)))


def _act_recip(nc, out, in_):
    """ACT-engine reciprocal (table approx; fine for the 2e-2 gate).
    bass guards func=Reciprocal behind a ValueError; emit Copy and patch."""
    ins = nc.scalar.activation(out=out, in_=in_, func=AF.Copy, bias=0.0)
    ins.ins.func = AF.Reciprocal
    return ins


# ---------------------------------------------------------------------------
# Kernel builder
# ---------------------------------------------------------------------------

def _bcast(ap, p):
    """[1,N] DRAM AP -> [p,N] partition-broadcast AP (stride-0 partition)."""
    return bass.AP(tensor=ap.tensor, offset=ap.offset,
                   ap=[[0, p]] + list(ap.ap[1:]))


def _bcast_rep(ap, p, rep):
    """[1,N] DRAM AP -> [p, rep*N] broadcast with free-dim repetition."""
    inner = list(ap.ap[1:])
    assert len(inner) == 1
    return bass.AP(tensor=ap.tensor, offset=ap.offset,
                   ap=[[0, p], [0, rep], inner[0]])


def _build(n_img):
    nc = bass.Bass("TRN2", target_bir_lowering=False)

    pb = nc.dram_tensor("pred_boxes", [n_img, NE, 4], F32, kind="ExternalInput")
    pl = nc.dram_tensor("pred_logits", [n_img, NE, NC1], F32, kind="ExternalInput")
    rol = nc.dram_tensor("pred_rel_obj_logits", [n_img, NR, NC1], F32, kind="ExternalInput")
    rsl = nc.dram_tensor("pred_rel_sub_logits", [n_img, NR, NC1], F32, kind="ExternalInput")
    rob = nc.dram_tensor("pred_rel_obj_box", [n_img, NR, 4], F32, kind="ExternalInput")
    rsb = nc.dram_tensor("pred_rel_sub_box", [n_img, NR, 4], F32, kind="ExternalInput")
    rv = nc.dram_tensor("pred_rel_vec", [n_img, NR, 4], F32, kind="ExternalInput")
    tsz = nc.dram_tensor("target_sizes", [n_img, 2], F32, kind="ExternalInput")
    out_s = nc.dram_tensor("out_sub", [n_img, NR, NE], F32, kind="ExternalOutput")
    out_o = nc.dram_tensor("out_obj", [n_img, NR, NE], F32, kind="ExternalOutput")

    with tile.TileContext(nc) as tc:
        with (
            tc.tile_pool(name="singles", bufs=1) as singles,
            tc.tile_pool(name="io", bufs=3) as io,
            tc.tile_pool(name="pre", bufs=4) as pre,
            tc.tile_pool(name="col", bufs=3) as col,
            tc.tile_pool(name="rep", bufs=2) as rep,
            tc.tile_pool(name="mm", bufs=2) as mm,
            tc.tile_pool(name="mp", bufs=1) as mp,
            tc.tile_pool(name="ps", bufs=2, space="PSUM") as ps,
            tc.tile_pool(name="psd", bufs=2, space="PSUM") as psd,
            tc.tile_pool(name="dr", bufs=2, space="DRAM") as dr,
        ):
            ident16 = singles.tile([128, 128], F16, tag="ident16")
            make_identity(nc, ident16)
            identf = singles.tile([128, 128], F32, tag="identf")
            make_identity(nc, identf)
            eps4 = singles.tile([128, 1], F32, tag="eps4")
            nc.vector.memset(eps4, 1e-4)
            neg1 = singles.tile([128, 1], F32, tag="neg1")
            nc.vector.memset(neg1, -1.0)

            for b in range(n_img):
                _build_image(nc, b, locals())
    _split_waits(nc)
    return nc


def _build_image(nc, b, env):
    io, pre, col, rep, mm, mp, ps, psd, dr = (env[k] for k in
        ("io", "pre", "col", "rep", "mm", "mp", "ps", "psd", "dr"))
    ident16, identf = env["ident16"], env["identf"]
    eps4 = env["eps4"]
    neg1 = env["neg1"]
    pb, pl, rol, rsl, rob, rsb, rv, tsz = (env[k] for k in
        ("pb", "pl", "rol", "rsl", "rob", "rsb", "rv", "tsz"))
    out_s, out_o = env["out_s"], env["out_o"]

    # image-level scalars: W, H broadcast to all partitions
    Wt = col.tile([128, 1], F32, tag="Wt")
    Ht = col.tile([128, 1], F32, tag="Ht")
    nc.sync.dma_start(out=Wt, in_=_bcast(tsz[b, 1:2], 128))
    nc.sync.dma_start(out=Ht, in_=_bcast(tsz[b, 0:1], 128))

    def box_cols(dram, tag, ncols):
        """cxcywh -> [P, NCH, ncols] f32: x0,y0,x1,y1,w,h,area (normalized)."""
        BT = io.tile([P, NCH, 4], F32, tag="BT_" + tag)
        nc.sync.dma_start(out=BT, in_=dram[b].rearrange("(j p) c -> p j c", p=P))
        PIX = col.tile([P, NCH, ncols], F32, tag=tag)
        nc.vector.scalar_tensor_tensor(
            out=PIX[:, :, 0:2], in0=BT[:, :, 2:4], scalar=-0.5,
            in1=BT[:, :, 0:2], op0=OP.mult, op1=OP.add)
        nc.vector.scalar_tensor_tensor(
            out=PIX[:, :, 2:4], in0=BT[:, :, 2:4], scalar=0.5,
            in1=BT[:, :, 0:2], op0=OP.mult, op1=OP.add)
        nc.vector.tensor_copy(out=PIX[:, :, 4:6], in_=BT[:, :, 2:4])
        nc.vector.tensor_tensor(out=PIX[:, :, 6:7], in0=BT[:, :, 2:3],
                                in1=BT[:, :, 3:4], op=OP.mult)
        return BT, PIX

    BT_E, PG = box_cols(pb, "PG", 10)       # cols 7=score 8=cxW 9=cyH
    nc.vector.tensor_scalar(out=PG[:, :, 8:9], in0=BT_E[:, :, 0:1],
                            scalar1=Wt[:P], scalar2=None, op0=OP.mult)
    nc.vector.tensor_scalar(out=PG[:, :, 9:10], in0=BT_E[:, :, 1:2],
                            scalar1=Ht[:P], scalar2=None, op0=OP.mult)

    _, PIX_S = box_cols(rsb, "PIX_S", 7)
    _, PIX_O = box_cols(rob, "PIX_O", 7)

    # VN = -rel_vec * (W,H,W,H) pixel (negated for use as ACT/TS bias)
    RVt = io.tile([P, NCH, 4], F32, tag="RVt")
    nc.sync.dma_start(out=RVt, in_=rv[b].rearrange("(j p) c -> p j c", p=P))
    SC4N = col.tile([128, 4], F32, tag="SC4N")
    nc.vector.tensor_scalar(out=SC4N[:, 0:1], in0=Wt, scalar1=-1.0,
                            scalar2=None, op0=OP.mult)
    nc.vector.tensor_scalar(out=SC4N[:, 1:2], in0=Ht, scalar1=-1.0,
                            scalar2=None, op0=OP.mult)
    nc.vector.tensor_copy(out=SC4N[:, 2:3], in_=SC4N[:, 0:1])
    nc.vector.tensor_copy(out=SC4N[:, 3:4], in_=SC4N[:, 1:2])
    VN = col.tile([P, NCH, 4], F32, tag="VN")
    for j in range(NCH):
        nc.vector.tensor_tensor(out=VN[:, j, :], in0=RVt[:, j, :],
                                in1=SC4N[:P], op=OP.mult)

    # ---- softmax + fp16 matmul operand packs (class-major via PE transpose)
    RHS_A = mm.tile([128, NE + 12], F16, tag="RHS_A")
    RHS_B = mm.tile([128, NE + 12], F16, tag="RHS_B")
    LS_A = mm.tile([128, NR + 12], F16, tag="LS_A")
    LS_B = mm.tile([128, NR + 12], F16, tag="LS_B")
    LO_A = mm.tile([128, NR + 12], F16, tag="LO_A")
    LO_B = mm.tile([128, NR + 12], F16, tag="LO_B")

    def pack(t, ldram, dA, dB):
        for j in range(NCH):
            L = io.tile([P, NC1], F32, tag="L")
            nc.sync.dma_start(out=L, in_=ldram[b, P * j:P * (j + 1), :])
            E = pre.tile([P, NC1], F32, tag="E")
            sumc = col.tile([P, 1], F32, tag="sumc")
            nc.scalar.activation(out=E, in_=L, func=AF.Exp, accum_out=sumc)
            r = col.tile([P, 1], F32, tag="r")
            nc.vector.reciprocal(r, sumc)
            PK = pre.tile([128, 256], F16, tag="PK")
            lp = nc.allow_low_precision(reason="fp16 pack norms, e2e checked")
            lp.__enter__()
            SQd = pre.tile([P, NCL], F32, tag="SQd")
            if t == "ent":
                nc.vector.tensor_scalar(out=PK[:P, 0:NCL], in0=E[:, :NCL],
                                        scalar1=r, scalar2=None, op0=OP.mult)
                nc.scalar.activation(out=SQd, in_=PK[:P, 0:NCL],
                                     func=AF.Square,
                                     accum_out=PK[:P, 151:152])
                nc.vector.tensor_reduce(out=PG[:, j, 7:8], in_=PK[:P, 0:NCL],
                                        axis=mybir.AxisListType.X, op=OP.max)
                nc.vector.memset(PK[:P, 150:151], 0.25)
            else:
                nc.vector.tensor_scalar(out=PK[:P, 0:NCL], in0=E[:, :NCL],
                                        scalar1=r, scalar2=-2.0, op0=OP.mult,
                                        op1=OP.mult)
                nc.scalar.activation(out=SQd, in_=PK[:P, 0:NCL],
                                     func=AF.Square,
                                     accum_out=PK[:P, 150:151])
                nc.vector.memset(PK[:P, 151:152], 1.0)
            lp.__exit__(None, None, None)
            TA = ps.tile([128, P], F16, tag="TT")
            nc.tensor.transpose(TA, PK[:P, 0:128], ident16[:P, :P])
            TB = ps.tile([24, P], F16, tag="TTB")
            nc.tensor.transpose(TB, PK[:P, 128:152], ident16[:P, :P])
            nc.vector.tensor_copy(out=dA[:, P * j:P * (j + 1)], in_=TA)
            nc.vector.tensor_copy(out=dB[0:24, P * j:P * (j + 1)], in_=TB)

    # ent pack first: the broadcast chain (PG -> EDRAM -> R tiles) only
    # depends on it, so the maps' chunk phase can start during the rel packs
    pack("ent", pl, RHS_A, RHS_B)

    # ---- entity rows -> DRAM -> broadcast tiles
    EROWS = rep.tile([8, NE], F16, tag="EROWS")
    EROWSF = rep.tile([10, NE], F32, tag="EROWSF")
    for j in range(NCH):
        TE24 = ps.tile([24, P], F32, tag="TTE")
        TE = TE24[0:10]
        nc.tensor.transpose(TE, PG[:, j, :], identf[:P, :P])
        nc.scalar.copy(out=EROWS[:, P * j:P * (j + 1)], in_=TE24[0:8])
        nc.scalar.copy(out=EROWSF[:, P * j:P * (j + 1)], in_=TE24[0:10])
    EDRAM = dr.tile([8, NE], F16, tag="EDRAM", name="EDRAM")
    nc.scalar.dma_start(out=EDRAM, in_=EROWS)
    EDRAMF = dr.tile([2, NE], F32, tag="EDRAMF", name="EDRAMF")
    nc.scalar.dma_start(out=EDRAMF, in_=EROWSF[8:10])

    names16 = ["X0R", "Y0R", "X1R", "Y1R", "WER", "HER", "AREAR"]
    R = {}
    for k, nm in enumerate(names16):
        R[nm] = rep.tile([128, NE], F16, tag=nm, name=nm)
        nc.sync.dma_start(out=R[nm], in_=_bcast(EDRAM[k:k + 1, :], 128))
    SCRQ = rep.tile([128, NQ], F16, tag="SCRQ", name="SCRQ")
    nc.sync.dma_start(out=SCRQ, in_=_bcast_rep(EDRAM[7:8, :], 128, NCH))
    CXR = rep.tile([128, NE], F32, tag="CXR", name="CXR")
    nc.sync.dma_start(out=CXR, in_=_bcast(EDRAMF[0:1, :], 128))
    CYR = rep.tile([128, NE], F32, tag="CYR", name="CYR")
    nc.sync.dma_start(out=CYR, in_=_bcast(EDRAMF[1:2, :], 128))

    pack("rs", rsl, LS_A, LS_B)
    pack("ro", rol, LO_A, LO_B)

    # ---- map pipeline: chunk phases of BOTH maps first, then quad phases
    # (1-stage software skew so each engine has ready work while the other
    # map's cross-engine chain drains)
    MAPS = (
        (LS_A, LS_B, PIX_S, 0, 1, out_s),
        (LO_A, LO_B, PIX_O, 2, 3, out_o),
    )

    def q(mi, tag, dt=F16):
        sfx = str(mi % 2)
        return mp.tile([P, NQ], dt, tag=tag + sfx, name=tag + sfx)

    st = {}
    for mi, (lA, lB, PIXR, vxc, vyc, odram) in enumerate(MAPS):
        m0x, m1x = q(mi, "m0x"), q(mi, "m1x")
        m0y, m1y = q(mi, "m0y"), q(mi, "m1y")
        axq, ayq = q(mi, "axq"), q(mi, "ayq")
        sqq = q(mi, "sqq")
        SAq = q(mi, "saq")
        SXq = q(mi, "sxq")
        SYq = q(mi, "syq")
        st[mi] = (m0x, m1x, m0y, m1y, axq, ayq, sqq, SAq, SXq, SYq)

        for j in range(NCH):
            sl = slice(NE * j, NE * (j + 1))
            rx0 = PIXR[:, j, 0:1]
            ry0 = PIXR[:, j, 1:2]
            rx1 = PIXR[:, j, 2:3]
            ry1 = PIXR[:, j, 3:4]
            nvx = VN[:, j, vxc:vxc + 1]
            nvy = VN[:, j, vyc:vyc + 1]
            rw = PIXR[:, j, 4:5]
            rh = PIXR[:, j, 5:6]
            rarea = PIXR[:, j, 6:7]

            D2 = psd.tile([P, NE], F32, tag="D2")
            nc.tensor.matmul(D2, lhsT=lA[:, P * j:P * (j + 1)],
                             rhs=RHS_A[:, 0:NE], start=True, stop=False)
            nc.tensor.matmul(D2, lhsT=lB[0:24, P * j:P * (j + 1)],
                             rhs=RHS_B[0:24, 0:NE], start=False, stop=True)
            nc.scalar.activation(out=sqq[:, sl], in_=D2, func=AF.Sqrt,
                                 bias=eps4[:P])
            nc.vector.tensor_scalar(out=m0x[:, sl], in0=R["X0R"][:P],
                                    scalar1=rx0, scalar2=None, op0=OP.max)
            nc.vector.tensor_scalar(out=m1x[:, sl], in0=R["X1R"][:P],
                                    scalar1=rx1, scalar2=None, op0=OP.min)
            nc.vector.tensor_scalar(out=m0y[:, sl], in0=R["Y0R"][:P],
                                    scalar1=ry0, scalar2=None, op0=OP.max)
            nc.vector.tensor_scalar(out=m1y[:, sl], in0=R["Y1R"][:P],
                                    scalar1=ry1, scalar2=None, op0=OP.min)
            nc.vector.tensor_scalar(out=axq[:, sl], in0=CXR[:P],
                                    scalar1=nvx, scalar2=None, op0=OP.add)
            nc.vector.tensor_scalar(out=ayq[:, sl], in0=CYR[:P],
                                    scalar1=nvy, scalar2=None, op0=OP.add)
            nc.gpsimd.tensor_scalar(out=SAq[:, sl], in0=R["AREAR"][:P],
                                    scalar1=rarea, scalar2=None, op0=OP.add)
            nc.gpsimd.tensor_scalar(out=SXq[:, sl], in0=R["WER"][:P],
                                    scalar1=rw, scalar2=None, op0=OP.add)
            nc.gpsimd.tensor_scalar(out=SYq[:, sl], in0=R["HER"][:P],
                                    scalar1=rh, scalar2=None, op0=OP.add)

    # quad phases of the two maps interleaved stage-by-stage: when one map's
    # chain waits on a cross-engine producer, the other map's same-stage op
    # is next in the queue.
    v = [dict(), dict()]
    for mi in range(2):
        (v[mi]["m0x"], v[mi]["m1x"], v[mi]["m0y"], v[mi]["m1y"],
         v[mi]["axq"], v[mi]["ayq"], v[mi]["sqq"],
         v[mi]["SAq"], v[mi]["SXq"], v[mi]["SYq"]) = st[mi]

    def stage(emit):
        for mi in range(2):
            emit(mi, v[mi])

    lp = nc.allow_low_precision(reason="fp16 map pipeline, e2e checked")
    lp.__enter__()

    def s_dx(mi, w):
        w["dxq"] = q(mi, "dxq")
        nc.vector.tensor_tensor(out=w["dxq"], in0=w["m1x"], in1=w["m0x"],
                                op=OP.subtract)
        w["dyq"] = q(mi, "dyq")
        nc.vector.tensor_tensor(out=w["dyq"], in0=w["m1y"], in1=w["m0y"],
                                op=OP.subtract)
    stage(s_dx)

    def s_relu(mi, w):
        w["rdx"] = q(mi, "m1x")  # m1x dead after dxq
        nc.scalar.activation(out=w["rdx"], in_=w["dxq"], func=AF.Relu)
        w["rdy"] = q(mi, "m1y")
        nc.scalar.activation(out=w["rdy"], in_=w["dyq"], func=AF.Relu)
    stage(s_relu)

    def s_I(mi, w):
        w["Iq"] = q(mi, "m0x")  # m0x dead after dxq
        nc.vector.tensor_tensor(out=w["Iq"], in0=w["rdx"], in1=w["rdy"],
                                op=OP.mult)
    stage(s_I)

    def s_U(mi, w):
        w["Uq"] = q(mi, "m0y")  # m0y dead after dyq
        nc.vector.tensor_tensor(out=w["Uq"], in0=w["SAq"], in1=w["Iq"],
                                op=OP.subtract)
        w["WCq"] = q(mi, "m1x")  # rdx dead after Iq
        nc.vector.tensor_tensor(out=w["WCq"], in0=w["SXq"], in1=w["dxq"],
                                op=OP.subtract)
        w["HCq"] = q(mi, "m1y")  # rdy dead after Iq
        nc.vector.tensor_tensor(out=w["HCq"], in0=w["SYq"], in1=w["dyq"],
                                op=OP.subtract)
    stage(s_U)

    def s_C(mi, w):
        w["Cq"] = q(mi, "saq")  # SAq dead after Uq
        nc.vector.tensor_tensor(out=w["Cq"], in0=w["WCq"], in1=w["HCq"],
                                op=OP.mult)
        nc.scalar.activation(out=w["axq"], in_=w["axq"], func=AF.Abs)
        nc.scalar.activation(out=w["ayq"], in_=w["ayq"], func=AF.Abs)
    stage(s_C)

    def s_L(mi, w):
        w["Lq"] = q(mi, "dxq")  # dxq dead after WCq
        nc.vector.tensor_tensor(out=w["Lq"], in0=w["axq"], in1=w["ayq"],
                                op=OP.add)
        w["t4"] = q(mi, "axq")  # axq dead after Lq
        nc.vector.tensor_tensor(out=w["t4"], in0=w["Iq"], in1=w["Uq"],
                                op=OP.subtract)
        w["U2"] = q(mi, "dyq")  # dyq dead after HCq
        nc.scalar.activation(out=w["U2"], in_=w["Uq"], func=AF.Square)
    stage(s_L)

    def s_N(mi, w):
        w["t5"] = q(mi, "ayq")  # ayq dead after Lq
        nc.vector.tensor_tensor(out=w["t5"], in0=w["Cq"], in1=w["t4"],
                                op=OP.mult)
        w["Nn"] = q(mi, "axq")  # t4 dead after t5
        nc.vector.tensor_tensor(out=w["Nn"], in0=w["t5"], in1=w["U2"],
                                op=OP.add)
        w["P1"] = q(mi, "m1x")  # WCq dead after Cq
        nc.vector.tensor_tensor(out=w["P1"], in0=w["Uq"], in1=w["Cq"],
                                op=OP.mult)
    stage(s_N)

    def s_den(mi, w):
        w["rN"] = q(mi, "sxq")  # SXq dead after WCq
        nc.vector.tensor_scalar(out=w["rN"], in0=w["Nn"], scalar1=0.0,
                                scalar2=None, op0=OP.max)
        w["LLq"] = q(mi, "syq")  # SYq dead after HCq
        nc.gpsimd.tensor_scalar(out=w["LLq"], in0=w["Lq"], scalar1=1.0,
                                scalar2=None, op0=OP.add)
        w["s1q"] = q(mi, "m0x")  # Iq dead after t4
        nc.gpsimd.tensor_scalar(out=w["s1q"], in0=w["sqq"], scalar1=1.0,
                                scalar2=None, op0=OP.add)
        w["d1q"] = q(mi, "m0y")  # Uq dead after P1
        nc.vector.tensor_tensor(out=w["d1q"], in0=w["LLq"], in1=w["s1q"],
                                op=OP.mult)
    stage(s_den)

    def s_D3(mi, w):
        w["D3"] = q(mi, "m1y")  # HCq dead after Cq
        nc.vector.tensor_tensor(out=w["D3"], in0=w["P1"], in1=w["d1q"],
                                op=OP.mult)
    stage(s_D3)

    def s_recip(mi, w):
        w["r3"] = q(mi, "saq")  # Cq dead after P1
        _act_recip(nc, w["r3"], w["D3"])
        w["tq"] = q(mi, "sqq")  # sqq dead after s1q
        nc.vector.tensor_tensor(out=w["tq"], in0=w["rN"], in1=SCRQ[:P],
                                op=OP.mult)
    stage(s_recip)

    lp.__exit__(None, None, None)

    def s_out(mi, w):
        odram = MAPS[mi][5]
        outq = mp.tile([P, NQ], F32, tag="outq" + str(mi % 2),
                       name="outq" + str(mi % 2))
        nc.gpsimd.tensor_tensor(out=outq, in0=w["tq"], in1=w["r3"],
                                op=OP.mult)
        for j in range(NCH):
            sl = slice(NE * j, NE * (j + 1))
            nc.gpsimd.dma_start(out=odram[b, P * j:P * (j + 1), :],
                                in_=outq[:, sl])
    stage(s_out)


class _CompiledKernel:
    """Compiled SPMD executable: jit built once, reusable across calls."""

    def __init__(self, nc, n_cores):
        import jax
        from jax.sharding import Mesh, PartitionSpec
        try:
            from jax.experimental.shard_map import shard_map
        except Exception:
            from jax.shard_map import shard_map
        from concourse import bass2jax
        from concourse.bass2jax import _bass_exec_p, install_neuronx_cc_hook

        install_neuronx_cc_hook()
        self.jax = jax
        self.n_cores = n_cores
        partition_name = (nc.partition_id_tensor.name
                          if nc.partition_id_tensor else None)
        in_names, out_names, out_avals, zero_outs = [], [], [], []
        for alloc in nc.m.functions[0].allocations:
            if not isinstance(alloc, mybir.MemoryLocationSet):
                continue
            name = alloc.memorylocations[0].name
            if alloc.kind == "ExternalInput":
                if name != partition_name:
                    in_names.append(name)
            elif alloc.kind == "ExternalOutput":
                shape = tuple(alloc.tensor_shape)
                dtype = mybir.dt.np(alloc.dtype)
                out_names.append(name)
                out_avals.append(jax.core.ShapedArray(shape, dtype))
                zero_outs.append(np.zeros(shape, dtype))
        self.in_names = in_names
        self.out_names = out_names
        self.out_avals = out_avals
        self.zero_outs = zero_outs
        all_in = in_names + out_names
        if partition_name is not None:
            all_in.append(partition_name)

        def _body(*args):
            operands = list(args)
            if partition_name is not None:
                operands.append(bass2jax.partition_id_tensor())
            return tuple(_bass_exec_p.bind(
                *operands,
                out_avals=tuple(out_avals),
                in_names=tuple(all_in),
                out_names=tuple(out_names),
                lowering_input_output_aliases=(),
                sim_require_finite=True,
                sim_require_nnan=True,
                nc=nc,
            ))

        devices = jax.devices()[:n_cores]
        self._mesh = Mesh(np.asarray(devices), ("core",))
        nin = len(in_names) + len(out_names)
        self._fn = jax.jit(
            shard_map(_body, mesh=self._mesh,
                      in_specs=(PartitionSpec("core"),) * nin,
                      out_specs=(PartitionSpec("core"),) * len(out_names),
                      check_rep=False),
            keep_unused=True)

    def run(self, in_maps):
        jax = self.jax
        n = self.n_cores
        per_core = [[np.asarray(m[nm]) for nm in self.in_names]
                    for m in in_maps]
        concat_in = [np.concatenate([per_core[c][i] for c in range(n)], axis=0)
                     for i in range(len(self.in_names))]
        concat_zero = [np.zeros((n * z.shape[0], *z.shape[1:]), z.dtype)
                       for z in self.zero_outs]
        outs = jax.block_until_ready(self._fn(*concat_in, *concat_zero))
        return [
            {nm: np.asarray(outs[i]).reshape(n, *self.out_avals[i].shape)[c]
             for i, nm in enumerate(self.out_names)}
            for c in range(n)
        ]


_CACHE = {}


def _get_nc():
    if "nc" not in _CACHE:
        _CACHE["nc"] = _build(N_IMG)
    return _CACHE["nc"]


def _get_ck():
    if "ck" not in _CACHE:
        _CACHE["ck"] = _CompiledKernel(_get_nc(), N_CORES)
    return _CACHE["ck"]


def kernel(pred_boxes, pred_logits, pred_rel_obj_logits, pred_rel_sub_logits,
           pred_rel_obj_box, pred_rel_sub_box, pred_rel_vec, target_sizes):
    inp = {
        "pred_boxes": np.ascontiguousarray(pred_boxes, dtype=np.float32),
        "pred_logits": np.ascontiguousarray(pred_logits, dtype=np.float32),
        "pred_rel_obj_logits": np.ascontiguousarray(pred_rel_obj_logits, dtype=np.float32),
        "pred_rel_sub_logits": np.ascontiguousarray(pred_rel_sub_logits, dtype=np.float32),
        "pred_rel_obj_box": np.ascontiguousarray(pred_rel_obj_box, dtype=np.float32),
        "pred_rel_sub_box": np.ascontiguousarray(pred_rel_sub_box, dtype=np.float32),
        "pred_rel_vec": np.ascontiguousarray(pred_rel_vec, dtype=np.float32),
        "target_sizes": np.ascontiguousarray(target_sizes, dtype=np.float32),
    }
    in_maps = [{k: v[c * N_IMG:(c + 1) * N_IMG] for k, v in inp.items()}
               for c in range(N_CORES)]
    res = None
    try:
        res = _get_ck().run(in_maps)
    except Exception:
        import time as _time
        _time.sleep(2.0)
        try:
            res = _get_ck().run(in_maps)
        except Exception:
            r = bass_utils.run_bass_kernel_spmd(
                _get_nc(), in_maps, core_ids=list(range(N_CORES)))
            res = r.results
    sub = np.concatenate([res[c]["out_sub"] for c in range(N_CORES)], axis=0)
    obj = np.concatenate([res[c]["out_obj"] for c in range(N_CORES)], axis=0)
    return sub, obj


# revision 46
# speedup vs baseline: 1.2270x; 1.2270x over previous
"""Trainium2 Bass kernel for nn_EntitiesIndexingHeadRuleBased (nms_detection).

kernel(**inputs) takes the FULL batch (B=64) and returns (sub_dist, obj_dist),
each [64, 500, 500] float32, matching the reference semantics:

  out_s[r,e] = relu(giou) * score_e / ((|vx-cx_e|+|vy-cy_e|+1) * (sqrt(d2)+1))

Sharding: pure data parallelism - 8 images per NeuronCore across 8 cores.

v2 redesign (vs f32 baseline):
  * fp16 intermediates unlock DVE 2x/4x perf modes (tensor_scalar 4x,
    tensor_tensor 2x); giou in N-form (relu(C*(I-U)+U^2) / (U*C)) with a
    single ACT-engine reciprocal covering the whole denominator.
  * GIoU evaluated on NORMALIZED boxes (scale-invariant), which keeps all
    box math in [0,2] and skips the W/H scaling of box tensors.
  * pixel centers stay f32; |cx - vx| subtraction runs as f32-input
    tensor_scalar (2x mode) to dodge catastrophic cancellation.
  * cls distance d2 >= 0 enforced structurally: both pack norms are
    sums of the same fp16-rounded prob vectors the matmul sees.
  * work spread across engines: DVE (tensor_scalar/tensor_tensor/divide),
    ACT (exp/sqrt/relu/square), Pool aka GpSimd (fold-ops + f32 output
    convert), PE (transposes + cdist matmuls), SyncE (DMA).
  * map-wide ops run at quad width [125, 2000] (4 chunks fused) to
    amortize per-op overheads.
"""
import sys
sys.path.insert(0, '/opt/trn_rl_repo')

import numpy as np
import bass_rust
import concourse.bass as bass
import concourse.tile as tile
import concourse.tile as tile_mod
from concourse import mybir
from concourse import bass_utils
from concourse.masks import make_identity
from concourse.tile import TileContext

F32 = mybir.dt.float32
F16 = mybir.dt.float16
AF = mybir.ActivationFunctionType
OP = mybir.AluOpType

B = 64
NE = 500
NR = 500
NC1 = 151
NCL = 150
P = 125
NCH = 4
NQ = NCH * NE          # quad width (2000)
N_CORES = 8
N_IMG = B // N_CORES

# ---------------------------------------------------------------------------
# Workarounds for the container's walrus: it rejects instructions carrying
# more than one sync-wait command ("Too many sync wait commands").
# ---------------------------------------------------------------------------

_MAXW = 1


def _patched_drain_and_barrier(self, tick_clock, wait_clock):
    ScopedClock = tile_mod.ScopedClock
    carrier = self.nc.sync.nop(nofuse=True)
    wait_clock.add_sem_waits(carrier.ins,
                             ScopedClock({None: tick_clock.global_clock}))
    si = carrier.ins.sync_info
    waits = list(si.on_wait) if si is not None else []
    if len(waits) > _MAXW:
        carrier.ins.sync_info = bass_rust.SyncInfo(
            on_wait=waits[:_MAXW], on_update=[])
        for i in range(_MAXW, len(waits), _MAXW):
            nop = self.nc.sync.nop(nofuse=True)
            nop.ins.sync_info = bass_rust.SyncInfo(
                on_wait=waits[i:i + _MAXW], on_update=[])
    self.nc.sync.drain()
    self.nc.all_engine_barrier()
    assert self.sems is not None
    popped = self.nc._tile_sem_poison_stack.pop()
    assert popped is self._sem_poison
    self.nc.clear_and_free_semaphores(list(self.sems.allocated().values()))
    self.nc.all_engine_barrier()


TileContext._drain_and_barrier = _patched_drain_and_barrier


def _split_waits(nc, maxw=_MAXW):
    """Hoist excess sync waits onto same-engine NoOps placed just before the
    offending instruction (engine streams execute in order)."""
    for fn in nc.m.functions:
        for blk in fn.blocks:
            newl = []
            changed = False
            for ins in blk.instructions:
                si = ins.sync_info
                waits = list(si.on_wait) if si is not None else []
                if len(waits) > maxw:
                    changed = True
                    carried, rest = waits[:-maxw], waits[-maxw:]
                    for i in range(0, len(carried), maxw):
                        nop = mybir.InstNoOp(
                            name=f"{ins.name}-sw{i}",
                            sync_info=mybir.SyncInfo(
                                on_wait=carried[i:i + maxw], on_update=[]),
                            bass_nofuse=True,
                            engine=ins.engine,
                        )
                        newl.append(nop)
                    ins.sync_info = mybir.SyncInfo(
                        on_wait=rest, on_update=list(si.on_update))
                newl.append(ins)
            if changed:
                blk.instructions = newl


# ---------------------------------------------------------------------------
# Custom fused DVE ops (registered into the process-local dve_ops registry;
# the per-NEFF DVE table is generated from these at compile time).
# ---------------------------------------------------------------------------

import numpy as _np
from concourse import dve_ops as _dve_ops
from concourse.dve_spec import (Spec, Src0, Src1, C0, C1, Zero, One,
                                maxx, minn, relu, lower,
                                _has_src1 as _spec_has_src1)
from concourse.dve_uop import DveOpSpec


def _register_op(name, spec):
    for o in _dve_ops.OPS:
        if o.name == name:
            return o
    row = _dve_ops._CUSTOM_DVE_ROW_BASE + len(_dve_ops.OPS)
    assert row < 0x20
    _dve_ops._SUB_OPCODE_FOR_NAME[name] = row
    shas = {}
    for ver in ("v3", "v4"):
        s = DveOpSpec(name=name, opcode=row, uops=lower(spec, ver=ver),
                      rd1_en=_spec_has_src1(spec))
        shas[ver] = s.sha(ver)
    op = _dve_ops.DveOp(name, spec, subdim=False, uops_sha=shas)
    _dve_ops.OPS.append(op)
    _dve_ops.CUSTOM_DVE_SPECS[name] = spec
    return op


# dx = min(in0, s0) - max(in1, s1)
DX_MM = _register_op("ANT_DX_MM", Spec(
    body=minn(Src0, C0) - maxx(Src1, C1),
    reference=lambda in0, in1, s0, s1, imm2:
        _np.minimum(in0, s0) - _np.maximum(in1, s1)))

# L = |in0 + s0| + |in1 + s1|
_a = Src0 + C0
_b = Src1 + C1
L1_AB = _register_op("ANT_L1_AB", Spec(
    body=maxx(_a, Zero - _a) + maxx(_b, Zero - _b),
    reference=lambda in0, in1, s0, s1, imm2:
        _np.abs(in0 + s0) + _np.abs(in1 + s1)))

# d1 = (in0 + 1) * (in1 + 1)
D1_FUSE = _register_op("ANT_D1_FUSE", Spec(
    body=(Src0 + One) * (Src1 + One),
    reference=lambda in0, in1, s0, s1, imm2: (in0 + 1.0) * (in1 + 1.0)))


def _act_recip(nc, out, in_):
    """ACT-engine reciprocal (table approx; fine for the 2e-2 gate).
    bass guards func=Reciprocal behind a ValueError; emit Copy and patch."""
    ins = nc.scalar.activation(out=out, in_=in_, func=AF.Copy, bias=0.0)
    ins.ins.func = AF.Reciprocal
    return ins


# ---------------------------------------------------------------------------
# Kernel builder
# ---------------------------------------------------------------------------

def _bcast(ap, p):
    """[1,N] DRAM AP -> [p,N] partition-broadcast AP (stride-0 partition)."""
    return bass.AP(tensor=ap.tensor, offset=ap.offset,
                   ap=[[0, p]] + list(ap.ap[1:]))


def _bcast_rep(ap, p, rep):
    """[1,N] DRAM AP -> [p, rep*N] broadcast with free-dim repetition."""
    inner = list(ap.ap[1:])
    assert len(inner) == 1
    return bass.AP(tensor=ap.tensor, offset=ap.offset,
                   ap=[[0, p], [0, rep], inner[0]])


def _build(n_img):
    nc = bass.Bass("TRN2", target_bir_lowering=False)

    pb = nc.dram_tensor("pred_boxes", [n_img, NE, 4], F32, kind="ExternalInput")
    pl = nc.dram_tensor("pred_logits", [n_img, NE, NC1], F32, kind="ExternalInput")
    rol = nc.dram_tensor("pred_rel_obj_logits", [n_img, NR, NC1], F32, kind="ExternalInput")
    rsl = nc.dram_tensor("pred_rel_sub_logits", [n_img, NR, NC1], F32, kind="ExternalInput")
    rob = nc.dram_tensor("pred_rel_obj_box", [n_img, NR, 4], F32, kind="ExternalInput")
    rsb = nc.dram_tensor("pred_rel_sub_box", [n_img, NR, 4], F32, kind="ExternalInput")
    rv = nc.dram_tensor("pred_rel_vec", [n_img, NR, 4], F32, kind="ExternalInput")
    tsz = nc.dram_tensor("target_sizes", [n_img, 2], F32, kind="ExternalInput")
    out_s = nc.dram_tensor("out_sub", [n_img, NR, NE], F32, kind="ExternalOutput")
    out_o = nc.dram_tensor("out_obj", [n_img, NR, NE], F32, kind="ExternalOutput")

    with tile.TileContext(nc) as tc:
        with (
            tc.tile_pool(name="singles", bufs=1) as singles,
            tc.tile_pool(name="io", bufs=3) as io,
            tc.tile_pool(name="pre", bufs=4) as pre,
            tc.tile_pool(name="col", bufs=3) as col,
            tc.tile_pool(name="rep", bufs=2) as rep,
            tc.tile_pool(name="mm", bufs=2) as mm,
            tc.tile_pool(name="mp", bufs=1) as mp,
            tc.tile_pool(name="ps", bufs=2, space="PSUM") as ps,
            tc.tile_pool(name="psd", bufs=2, space="PSUM") as psd,
            tc.tile_pool(name="dr", bufs=2, space="DRAM") as dr,
        ):
            ident16 = singles.tile([128, 128], F16, tag="ident16")
            make_identity(nc, ident16)
            identf = singles.tile([128, 128], F32, tag="identf")
            make_identity(nc, identf)
            eps4 = singles.tile([128, 1], F32, tag="eps4")
            nc.vector.memset(eps4, 1e-4)
            neg1 = singles.tile([128, 1], F32, tag="neg1")
            nc.vector.memset(neg1, -1.0)

            for b in range(n_img):
                _build_image(nc, b, locals())
    _split_waits(nc)
    return nc


def _build_image(nc, b, env):
    io, pre, col, rep, mm, mp, ps, psd, dr = (env[k] for k in
        ("io", "pre", "col", "rep", "mm", "mp", "ps", "psd", "dr"))
    ident16, identf = env["ident16"], env["identf"]
    eps4 = env["eps4"]
    neg1 = env["neg1"]
    pb, pl, rol, rsl, rob, rsb, rv, tsz = (env[k] for k in
        ("pb", "pl", "rol", "rsl", "rob", "rsb", "rv", "tsz"))
    out_s, out_o = env["out_s"], env["out_o"]

    # image-level scalars: W, H broadcast to all partitions
    Wt = col.tile([128, 1], F32, tag="Wt")
    Ht = col.tile([128, 1], F32, tag="Ht")
    nc.sync.dma_start(out=Wt, in_=_bcast(tsz[b, 1:2], 128))
    nc.sync.dma_start(out=Ht, in_=_bcast(tsz[b, 0:1], 128))

    def box_cols(dram, tag, ncols):
        """cxcywh -> [P, NCH, ncols] f32: x0,y0,x1,y1,w,h,area (normalized)."""
        BT = io.tile([P, NCH, 4], F32, tag="BT_" + tag)
        nc.sync.dma_start(out=BT, in_=dram[b].rearrange("(j p) c -> p j c", p=P))
        PIX = col.tile([P, NCH, ncols], F32, tag=tag)
        nc.vector.scalar_tensor_tensor(
            out=PIX[:, :, 0:2], in0=BT[:, :, 2:4], scalar=-0.5,
            in1=BT[:, :, 0:2], op0=OP.mult, op1=OP.add)
        nc.vector.scalar_tensor_tensor(
            out=PIX[:, :, 2:4], in0=BT[:, :, 2:4], scalar=0.5,
            in1=BT[:, :, 0:2], op0=OP.mult, op1=OP.add)
        nc.vector.tensor_copy(out=PIX[:, :, 4:6], in_=BT[:, :, 2:4])
        nc.vector.tensor_tensor(out=PIX[:, :, 6:7], in0=BT[:, :, 2:3],
                                in1=BT[:, :, 3:4], op=OP.mult)
        return BT, PIX

    BT_E, PG = box_cols(pb, "PG", 10)       # cols 7=score 8=cxW 9=cyH
    nc.vector.tensor_scalar(out=PG[:, :, 8:9], in0=BT_E[:, :, 0:1],
                            scalar1=Wt[:P], scalar2=None, op0=OP.mult)
    nc.vector.tensor_scalar(out=PG[:, :, 9:10], in0=BT_E[:, :, 1:2],
                            scalar1=Ht[:P], scalar2=None, op0=OP.mult)

    _, PIX_S = box_cols(rsb, "PIX_S", 7)
    _, PIX_O = box_cols(rob, "PIX_O", 7)

    # VN = -rel_vec * (W,H,W,H) pixel (negated for use as ACT/TS bias)
    RVt = io.tile([P, NCH, 4], F32, tag="RVt")
    nc.sync.dma_start(out=RVt, in_=rv[b].rearrange("(j p) c -> p j c", p=P))
    SC4N = col.tile([128, 4], F32, tag="SC4N")
    nc.vector.tensor_scalar(out=SC4N[:, 0:1], in0=Wt, scalar1=-1.0,
                            scalar2=None, op0=OP.mult)
    nc.vector.tensor_scalar(out=SC4N[:, 1:2], in0=Ht, scalar1=-1.0,
                            scalar2=None, op0=OP.mult)
    nc.vector.tensor_copy(out=SC4N[:, 2:3], in_=SC4N[:, 0:1])
    nc.vector.tensor_copy(out=SC4N[:, 3:4], in_=SC4N[:, 1:2])
    VN = col.tile([P, NCH, 4], F32, tag="VN")
    for j in range(NCH):
        nc.vector.tensor_tensor(out=VN[:, j, :], in0=RVt[:, j, :],
                                in1=SC4N[:P], op=OP.mult)

    # ---- softmax + fp16 matmul operand packs (class-major via PE transpose)
    RHS_A = mm.tile([128, NE + 12], F16, tag="RHS_A")
    RHS_B = mm.tile([128, NE + 12], F16, tag="RHS_B")
    LS_A = mm.tile([128, NR + 12], F16, tag="LS_A")
    LS_B = mm.tile([128, NR + 12], F16, tag="LS_B")
    LO_A = mm.tile([128, NR + 12], F16, tag="LO_A")
    LO_B = mm.tile([128, NR + 12], F16, tag="LO_B")

    def pack(t, ldram, dA, dB):
        for j in range(NCH):
            L = io.tile([P, NC1], F32, tag="L")
            nc.sync.dma_start(out=L, in_=ldram[b, P * j:P * (j + 1), :])
            E = pre.tile([P, NC1], F32, tag="E")
            sumc = col.tile([P, 1], F32, tag="sumc")
            nc.scalar.activation(out=E, in_=L, func=AF.Exp, accum_out=sumc)
            r = col.tile([P, 1], F32, tag="r")
            nc.vector.reciprocal(r, sumc)
            PK = pre.tile([128, 256], F16, tag="PK")
            lp = nc.allow_low_precision(reason="fp16 pack norms, e2e checked")
            lp.__enter__()
            SQd = pre.tile([P, NCL], F32, tag="SQd")
            if t == "ent":
                nc.vector.tensor_scalar(out=PK[:P, 0:NCL], in0=E[:, :NCL],
                                        scalar1=r, scalar2=None, op0=OP.mult)
                nc.scalar.activation(out=SQd, in_=PK[:P, 0:NCL],
                                     func=AF.Square,
                                     accum_out=PK[:P, 151:152])
                nc.vector.tensor_reduce(out=PG[:, j, 7:8], in_=PK[:P, 0:NCL],
                                        axis=mybir.AxisListType.X, op=OP.max)
                nc.vector.memset(PK[:P, 150:151], 0.25)
            else:
                nc.vector.tensor_scalar(out=PK[:P, 0:NCL], in0=E[:, :NCL],
                                        scalar1=r, scalar2=-2.0, op0=OP.mult,
                                        op1=OP.mult)
                nc.scalar.activation(out=SQd, in_=PK[:P, 0:NCL],
                                     func=AF.Square,
                                     accum_out=PK[:P, 150:151])
                nc.vector.memset(PK[:P, 151:152], 1.0)
            lp.__exit__(None, None, None)
            TA = ps.tile([128, P], F16, tag="TT")
            nc.tensor.transpose(TA, PK[:P, 0:128], ident16[:P, :P])
            TB = ps.tile([24, P], F16, tag="TTB")
            nc.tensor.transpose(TB, PK[:P, 128:152], ident16[:P, :P])
            nc.vector.tensor_copy(out=dA[:, P * j:P * (j + 1)], in_=TA)
            nc.vector.tensor_copy(out=dB[0:24, P * j:P * (j + 1)], in_=TB)

    # ent pack first: the broadcast chain (PG -> EDRAM -> R tiles) only
    # depends on it, so the maps' chunk phase can start during the rel packs
    pack("ent", pl, RHS_A, RHS_B)

    # ---- entity rows -> DRAM -> broadcast tiles
    EROWS = rep.tile([8, NE], F16, tag="EROWS")
    EROWSF = rep.tile([10, NE], F32, tag="EROWSF")
    for j in range(NCH):
        TE24 = ps.tile([24, P], F32, tag="TTE")
        TE = TE24[0:10]
        nc.tensor.transpose(TE, PG[:, j, :], identf[:P, :P])
        nc.scalar.copy(out=EROWS[:, P * j:P * (j + 1)], in_=TE24[0:8])
        nc.scalar.copy(out=EROWSF[:, P * j:P * (j + 1)], in_=TE24[0:10])
    EDRAM = dr.tile([8, NE], F16, tag="EDRAM", name="EDRAM")
    nc.scalar.dma_start(out=EDRAM, in_=EROWS)
    EDRAMF = dr.tile([2, NE], F32, tag="EDRAMF", name="EDRAMF")
    nc.scalar.dma_start(out=EDRAMF, in_=EROWSF[8:10])

    names16 = ["X0R", "Y0R", "X1R", "Y1R", "WER", "HER", "AREAR"]
    R = {}
    for k, nm in enumerate(names16):
        R[nm] = rep.tile([128, NE], F16, tag=nm, name=nm)
        nc.sync.dma_start(out=R[nm], in_=_bcast(EDRAM[k:k + 1, :], 128))
    SCRQ = rep.tile([128, NQ], F16, tag="SCRQ", name="SCRQ")
    nc.sync.dma_start(out=SCRQ, in_=_bcast_rep(EDRAM[7:8, :], 128, NCH))
    CXR = rep.tile([128, NE], F32, tag="CXR", name="CXR")
    nc.sync.dma_start(out=CXR, in_=_bcast(EDRAMF[0:1, :], 128))
    CYR = rep.tile([128, NE], F32, tag="CYR", name="CYR")
    nc.sync.dma_start(out=CYR, in_=_bcast(EDRAMF[1:2, :], 128))

    pack("rs", rsl, LS_A, LS_B)
    pack("ro", rol, LO_A, LO_B)

    # ---- map pipeline: chunk phases of BOTH maps first, then quad phases
    # (1-stage software skew so each engine has ready work while the other
    # map's cross-engine chain drains)
    MAPS = (
        (LS_A, LS_B, PIX_S, 0, 1, out_s),
        (LO_A, LO_B, PIX_O, 2, 3, out_o),
    )

    def q(mi, tag, dt=F16):
        sfx = str(mi % 2)
        return mp.tile([P, NQ], dt, tag=tag + sfx, name=tag + sfx)

    st = {}
    for mi, (lA, lB, PIXR, vxc, vyc, odram) in enumerate(MAPS):
        m0x, m1x = q(mi, "m0x"), q(mi, "m1x")
        m0y, m1y = q(mi, "m0y"), q(mi, "m1y")
        axq, ayq = q(mi, "axq"), q(mi, "ayq")
        sqq = q(mi, "sqq")
        SAq = q(mi, "saq")
        SXq = q(mi, "sxq")
        SYq = q(mi, "syq")
        st[mi] = (m0x, m1x, m0y, m1y, axq, ayq, sqq, SAq, SXq, SYq)

        for j in range(NCH):
            sl = slice(NE * j, NE * (j + 1))
            rx0 = PIXR[:, j, 0:1]
            ry0 = PIXR[:, j, 1:2]
            rx1 = PIXR[:, j, 2:3]
            ry1 = PIXR[:, j, 3:4]
            nvx = VN[:, j, vxc:vxc + 1]
            nvy = VN[:, j, vyc:vyc + 1]
            rw = PIXR[:, j, 4:5]
            rh = PIXR[:, j, 5:6]
            rarea = PIXR[:, j, 6:7]

            D2 = psd.tile([P, NE], F32, tag="D2")
            nc.tensor.matmul(D2, lhsT=lA[:, P * j:P * (j + 1)],
                             rhs=RHS_A[:, 0:NE], start=True, stop=False)
            nc.tensor.matmul(D2, lhsT=lB[0:24, P * j:P * (j + 1)],
                             rhs=RHS_B[0:24, 0:NE], start=False, stop=True)
            nc.scalar.activation(out=sqq[:, sl], in_=D2, func=AF.Sqrt,
                                 bias=eps4[:P])
            nc.vector.tensor_scalar(out=m0x[:, sl], in0=R["X0R"][:P],
                                    scalar1=rx0, scalar2=None, op0=OP.max)
            nc.vector.tensor_scalar(out=m1x[:, sl], in0=R["X1R"][:P],
                                    scalar1=rx1, scalar2=None, op0=OP.min)
            nc.vector.tensor_scalar(out=m0y[:, sl], in0=R["Y0R"][:P],
                                    scalar1=ry0, scalar2=None, op0=OP.max)
            nc.vector.tensor_scalar(out=m1y[:, sl], in0=R["Y1R"][:P],
                                    scalar1=ry1, scalar2=None, op0=OP.min)
            nc.vector.tensor_scalar(out=axq[:, sl], in0=CXR[:P],
                                    scalar1=nvx, scalar2=None, op0=OP.add)
            nc.vector.tensor_scalar(out=ayq[:, sl], in0=CYR[:P],
                                    scalar1=nvy, scalar2=None, op0=OP.add)
            nc.gpsimd.tensor_scalar(out=SAq[:, sl], in0=R["AREAR"][:P],
                                    scalar1=rarea, scalar2=None, op0=OP.add)
            nc.gpsimd.tensor_scalar(out=SXq[:, sl], in0=R["WER"][:P],
                                    scalar1=rw, scalar2=None, op0=OP.add)
            nc.gpsimd.tensor_scalar(out=SYq[:, sl], in0=R["HER"][:P],
                                    scalar1=rh, scalar2=None, op0=OP.add)

    # quad phases of the two maps interleaved stage-by-stage: when one map's
    # chain waits on a cross-engine producer, the other map's same-stage op
    # is next in the queue.
    v = [dict(), dict()]
    for mi in range(2):
        (v[mi]["m0x"], v[mi]["m1x"], v[mi]["m0y"], v[mi]["m1y"],
         v[mi]["axq"], v[mi]["ayq"], v[mi]["sqq"],
         v[mi]["SAq"], v[mi]["SXq"], v[mi]["SYq"]) = st[mi]

    def stage(emit):
        for mi in range(2):
            emit(mi, v[mi])

    lp = nc.allow_low_precision(reason="fp16 map pipeline, e2e checked")
    lp.__enter__()

    def s_dx(mi, w):
        w["dxq"] = q(mi, "dxq")
        nc.vector.tensor_tensor(out=w["dxq"], in0=w["m1x"], in1=w["m0x"],
                                op=OP.subtract)
        w["dyq"] = q(mi, "dyq")
        nc.vector.tensor_tensor(out=w["dyq"], in0=w["m1y"], in1=w["m0y"],
                                op=OP.subtract)
    stage(s_dx)

    def s_relu(mi, w):
        w["rdx"] = q(mi, "m1x")  # m1x dead after dxq
        nc.scalar.activation(out=w["rdx"], in_=w["dxq"], func=AF.Relu)
        w["rdy"] = q(mi, "m1y")
        nc.scalar.activation(out=w["rdy"], in_=w["dyq"], func=AF.Relu)
    stage(s_relu)

    def s_I(mi, w):
        w["Iq"] = q(mi, "m0x")  # m0x dead after dxq
        nc.vector.tensor_tensor(out=w["Iq"], in0=w["rdx"], in1=w["rdy"],
                                op=OP.mult)
    stage(s_I)

    def s_U(mi, w):
        w["Uq"] = q(mi, "m0y")  # m0y dead after dyq
        nc.vector.tensor_tensor(out=w["Uq"], in0=w["SAq"], in1=w["Iq"],
                                op=OP.subtract)
        w["WCq"] = q(mi, "m1x")  # rdx dead after Iq
        nc.vector.tensor_tensor(out=w["WCq"], in0=w["SXq"], in1=w["dxq"],
                                op=OP.subtract)
        w["HCq"] = q(mi, "m1y")  # rdy dead after Iq
        nc.vector.tensor_tensor(out=w["HCq"], in0=w["SYq"], in1=w["dyq"],
                                op=OP.subtract)
    stage(s_U)

    def s_C(mi, w):
        w["Cq"] = q(mi, "saq")  # SAq dead after Uq
        nc.vector.tensor_tensor(out=w["Cq"], in0=w["WCq"], in1=w["HCq"],
                                op=OP.mult)
        nc.scalar.activation(out=w["axq"], in_=w["axq"], func=AF.Abs)
        nc.scalar.activation(out=w["ayq"], in_=w["ayq"], func=AF.Abs)
    stage(s_C)

    def s_L(mi, w):
        w["Lq"] = q(mi, "dxq")  # dxq dead after WCq
        nc.vector.tensor_tensor(out=w["Lq"], in0=w["axq"], in1=w["ayq"],
                                op=OP.add)
        w["t4"] = q(mi, "axq")  # axq dead after Lq
        nc.vector.tensor_tensor(out=w["t4"], in0=w["Iq"], in1=w["Uq"],
                                op=OP.subtract)
        w["U2"] = q(mi, "dyq")  # dyq dead after HCq
        nc.scalar.activation(out=w["U2"], in_=w["Uq"], func=AF.Square)
    stage(s_L)

    def s_N(mi, w):
        w["t5"] = q(mi, "ayq")  # ayq dead after Lq
        nc.vector.tensor_tensor(out=w["t5"], in0=w["Cq"], in1=w["t4"],
                                op=OP.mult)
        w["Nn"] = q(mi, "axq")  # t4 dead after t5
        nc.vector.tensor_tensor(out=w["Nn"], in0=w["t5"], in1=w["U2"],
                                op=OP.add)
        w["P1"] = q(mi, "m1x")  # WCq dead after Cq
        nc.vector.tensor_tensor(out=w["P1"], in0=w["Uq"], in1=w["Cq"],
                                op=OP.mult)
    stage(s_N)

    def s_den(mi, w):
        w["rN"] = q(mi, "sxq")  # SXq dead after WCq
        nc.gpsimd.tensor_scalar(out=w["rN"], in0=w["Nn"], scalar1=0.0,
                                scalar2=None, op0=OP.max)
        w["LLq"] = q(mi, "syq")  # SYq dead after HCq
        nc.gpsimd.tensor_scalar(out=w["LLq"], in0=w["Lq"], scalar1=1.0,
                                scalar2=None, op0=OP.add)
        w["s1q"] = q(mi, "m0x")  # Iq dead after t4
        nc.gpsimd.tensor_scalar(out=w["s1q"], in0=w["sqq"], scalar1=1.0,
                                scalar2=None, op0=OP.add)
        w["d1q"] = q(mi, "m0y")  # Uq dead after P1
        nc.vector.tensor_tensor(out=w["d1q"], in0=w["LLq"], in1=w["s1q"],
                                op=OP.mult)
    stage(s_den)

    def s_D3(mi, w):
        w["D3"] = q(mi, "m1y")  # HCq dead after Cq
        nc.vector.tensor_tensor(out=w["D3"], in0=w["P1"], in1=w["d1q"],
                                op=OP.mult)
    stage(s_D3)

    def s_recip(mi, w):
        w["r3"] = q(mi, "saq")  # Cq dead after P1
        _act_recip(nc, w["r3"], w["D3"])
        w["tq"] = q(mi, "sqq")  # sqq dead after s1q
        nc.vector.tensor_tensor(out=w["tq"], in0=w["rN"], in1=SCRQ[:P],
                                op=OP.mult)
    stage(s_recip)

    lp.__exit__(None, None, None)

    def s_out(mi, w):
        odram = MAPS[mi][5]
        outq = mp.tile([P, NQ], F32, tag="outq" + str(mi % 2),
                       name="outq" + str(mi % 2))
        nc.gpsimd.tensor_tensor(out=outq, in0=w["tq"], in1=w["r3"],
                                op=OP.mult)
        for j in range(NCH):
            sl = slice(NE * j, NE * (j + 1))
            nc.gpsimd.dma_start(out=odram[b, P * j:P * (j + 1), :],
                                in_=outq[:, sl])
    stage(s_out)


class _CompiledKernel:
    """Compiled SPMD executable: jit built once, reusable across calls."""

    def __init__(self, nc, n_cores):
        import jax
        from jax.sharding import Mesh, PartitionSpec
        try:
            from jax.experimental.shard_map import shard_map
        except Exception:
            from jax.shard_map import shard_map
        from concourse import bass2jax
        from concourse.bass2jax import _bass_exec_p, install_neuronx_cc_hook

        install_neuronx_cc_hook()
        self.jax = jax
        self.n_cores = n_cores
        partition_name = (nc.partition_id_tensor.name
                          if nc.partition_id_tensor else None)
        in_names, out_names, out_avals, zero_outs = [], [], [], []
        for alloc in nc.m.functions[0].allocations:
            if not isinstance(alloc, mybir.MemoryLocationSet):
                continue
            name = alloc.memorylocations[0].name
            if alloc.kind == "ExternalInput":
                if name != partition_name:
                    in_names.append(name)
            elif alloc.kind == "ExternalOutput":
                shape = tuple(alloc.tensor_shape)
                dtype = mybir.dt.np(alloc.dtype)
                out_names.append(name)
                out_avals.append(jax.core.ShapedArray(shape, dtype))
                zero_outs.append(np.zeros(shape, dtype))
        self.in_names = in_names
        self.out_names = out_names
        self.out_avals = out_avals
        self.zero_outs = zero_outs
        all_in = in_names + out_names
        if partition_name is not None:
            all_in.append(partition_name)

        def _body(*args):
            operands = list(args)
            if partition_name is not None:
                operands.append(bass2jax.partition_id_tensor())
            return tuple(_bass_exec_p.bind(
                *operands,
                out_avals=tuple(out_avals),
                in_names=tuple(all_in),
                out_names=tuple(out_names),
                lowering_input_output_aliases=(),
                sim_require_finite=True,
                sim_require_nnan=True,
                nc=nc,
            ))

        devices = jax.devices()[:n_cores]
        self._mesh = Mesh(np.asarray(devices), ("core",))
        nin = len(in_names) + len(out_names)
        self._fn = jax.jit(
            shard_map(_body, mesh=self._mesh,
                      in_specs=(PartitionSpec("core"),) * nin,
                      out_specs=(PartitionSpec("core"),) * len(out_names),
                      check_rep=False),
            keep_unused=True)

    def run(self, in_maps):
        jax = self.jax
        n = self.n_cores
        per_core = [[np.asarray(m[nm]) for nm in self.in_names]
                    for m in in_maps]
        concat_in = [np.concatenate([per_core[c][i] for c in range(n)], axis=0)
                     for i in range(len(self.in_names))]
        concat_zero = [np.zeros((n * z.shape[0], *z.shape[1:]), z.dtype)
                       for z in self.zero_outs]
        outs = jax.block_until_ready(self._fn(*concat_in, *concat_zero))
        return [
            {nm: np.asarray(outs[i]).reshape(n, *self.out_avals[i].shape)[c]
             for i, nm in enumerate(self.out_names)}
            for c in range(n)
        ]


_CACHE = {}


def _get_nc():
    if "nc" not in _CACHE:
        _CACHE["nc"] = _build(N_IMG)
    return _CACHE["nc"]


def _get_ck():
    if "ck" not in _CACHE:
        _CACHE["ck"] = _CompiledKernel(_get_nc(), N_CORES)
    return _CACHE["ck"]


def kernel(pred_boxes, pred_logits, pred_rel_obj_logits, pred_rel_sub_logits,
           pred_rel_obj_box, pred_rel_sub_box, pred_rel_vec, target_sizes):
    inp = {
        "pred_boxes": np.ascontiguousarray(pred_boxes, dtype=np.float32),
        "pred_logits": np.ascontiguousarray(pred_logits, dtype=np.float32),
        "pred_rel_obj_logits": np.ascontiguousarray(pred_rel_obj_logits, dtype=np.float32),
        "pred_rel_sub_logits": np.ascontiguousarray(pred_rel_sub_logits, dtype=np.float32),
        "pred_rel_obj_box": np.ascontiguousarray(pred_rel_obj_box, dtype=np.float32),
        "pred_rel_sub_box": np.ascontiguousarray(pred_rel_sub_box, dtype=np.float32),
        "pred_rel_vec": np.ascontiguousarray(pred_rel_vec, dtype=np.float32),
        "target_sizes": np.ascontiguousarray(target_sizes, dtype=np.float32),
    }
    in_maps = [{k: v[c * N_IMG:(c + 1) * N_IMG] for k, v in inp.items()}
               for c in range(N_CORES)]
    res = None
    try:
        res = _get_ck().run(in_maps)
    except Exception:
        import time as _time
        _time.sleep(2.0)
        try:
            res = _get_ck().run(in_maps)
        except Exception:
            r = bass_utils.run_bass_kernel_spmd(
                _get_nc(), in_maps, core_ids=list(range(N_CORES)))
            res = r.results
    sub = np.concatenate([res[c]["out_sub"] for c in range(N_CORES)], axis=0)
    obj = np.concatenate([res[c]["out_obj"] for c in range(N_CORES)], axis=0)
    return sub, obj
